# revision 38
# baseline (speedup 1.0000x reference)
"""Distributed HGNN+ convolution for 8 Trainium2 NeuronCores (Bass/Tile).

Math (dense hypergraph incidence H [N_V, N_E], features X [N_V, C]):
    Xt  = X @ W.T + b                    # theta
    Xe  = (H.T @ Xt) * 1/colsum(H)       # V2E mean aggregation
    Xv  = (H @ Xe)   * 1/rowsum(H)       # E2V mean aggregation
    out = relu(Xv)

Distribution: vertex rows are sharded across the 8 cores. Each core
computes theta on its vertex shard, a partial V2E GEMM, an AllReduce of
the partial edge features, then a fully row-parallel E2V GEMM over its
own vertex rows.

Both degree scalings are folded into the E2V operand on the host:
    htp2 = diag(4/rowsum) @ H @ diag(1/colsum)
so the device graph is three plain GEMMs + ReLU: every matmul streams a
full 512-wide PSUM bank and no on-device reciprocals/rescales exist.
(The 4x / 1/4 pair keeps the fp8 wire format in range, see below.)

The AllReduce runs in fp8e4 (half the SDMA/HBM footprint) in two
asymmetric chunks sized to hide it entirely: chunk 0 (40 edge tiles)
fires at ~62% of V2E and completes under the remaining V2E compute;
chunk 1 (24 tiles) fires at V2E end and completes under E2V's chunk-0
compute. E2V is chunk-major with a bf16 SBUF accumulator; the final
chunk seeds its PSUM accumulation group from the accumulator so the
epilogue is a single ReLU from PSUM.

During a concurrent collective, model-queue DMA throughput collapses
~10x, so both sides of the V2E pipeline carry deep SBUF rings (15 hs
input panels, 24 fp8 output staging tiles) to ride out the window.
GEMM compute is bf16 with fp32 PSUM accumulation; rel-err lands ~4.5e-3
against the fp32 reference, within the 2e-2 envelope.
"""

import contextlib

import numpy as np
import ml_dtypes

BF16 = ml_dtypes.bfloat16

# Problem shape (hardcoded per contract).
N_V, N_E, CH, NCORES = 16384, 8192, 512, 8


def _full_cfg():
    # Asymmetric AllReduce chunks (in 128-edge tiles): chunk 0 fires at
    # ~62% of V2E and hides under its tail; chunk 1 hides under E2V
    # chunk-0 compute (~170us vs ~55-115us collective).
    return dict(n_v=N_V, n_e=N_E, ch=CH, ncores=NCORES, chunks=(40, 24))


def build_graph(tc, io, cfg):
    """Emit the Tile IR. io: dict of DRAM APs: hsp, htp, xta, wtb, out."""
    from concourse import mybir

    nc = tc.nc
    f32 = mybir.dt.float32
    bf16 = mybir.dt.bfloat16
    f8 = mybir.dt.float8e4
    Relu = mybir.ActivationFunctionType.Relu

    n_v, n_e, ch, ncores = cfg["n_v"], cfg["n_e"], cfg["ch"], cfg["ncores"]
    chunks = list(cfg["chunks"])    # AR chunk sizes in 128-edge tiles
    nch = len(chunks)
    cstart = [sum(chunks[:i]) for i in range(nch + 1)]  # tile offsets
    VS = n_v // ncores      # vertices per core
    KV = VS // 128          # vertex 128-tiles per core
    EM = n_e // 128         # edge 128-tiles (global)
    CK = ch // 128          # theta contraction tiles over in-channels
    assert cstart[nch] == EM
    rg = [list(range(ncores))]

    hsp, htp, xta, wtb, out = io["hsp"], io["htp"], io["xta"], io["wtb"], io["out"]

    with contextlib.ExitStack() as ctx:
        xt_pool = ctx.enter_context(tc.tile_pool(name="xt_pool", bufs=1))
        xe_pool = ctx.enter_context(tc.tile_pool(name="xe_pool", bufs=1))
        acc_pool = ctx.enter_context(tc.tile_pool(name="acc_pool", bufs=1))
        sb_out = ctx.enter_context(tc.tile_pool(name="sb_out", bufs=2))
        # Deep fp8 staging ring: during a concurrent collective, model-queue
        # DMA throughput collapses ~10x, so V2E's arin stores crawl. The
        # ring lets ~24 edge tiles' outputs buffer in SBUF (cheap at 0.5KB/
        # partition) so the matmul stream never blocks on the store side.
        ar_pool = ctx.enter_context(tc.tile_pool(name="ar_pool", bufs=24))
        psum = ctx.enter_context(tc.tile_pool(name="psum", bufs=4, space="PSUM"))
        dram = ctx.enter_context(tc.tile_pool(name="dram", bufs=1, space="DRAM"))

        # ---- theta: Xt = X.T @ W.T + (0.25-ones) @ b, kept in SBUF as KV
        # tiles of [128 v, ch]. The bias is a rank-1 (K=1) matmul so no
        # zero-padded contraction tile is loaded. Per-k-tile DMAs so the
        # first theta matmul only waits for the first slices. The pool is
        # scoped to this phase so its SBUF is recycled into the hs ring.
        xt_all = xt_pool.tile([128, KV * ch], bf16)
        with tc.tile_pool(name="theta_in", bufs=1) as theta_in:
            xta_sb = theta_in.tile([128, CK * VS], bf16)
            wtb_sb = theta_in.tile([128, CK * ch], bf16)
            one_sb = theta_in.tile([1, VS], bf16)
            bias_sb = theta_in.tile([1, ch], bf16)
            nc.sync.dma_start(bias_sb, wtb[CK * 128 : CK * 128 + 1, :])
            for kt in range(CK):
                nc.sync.dma_start(
                    wtb_sb[:, kt * ch : (kt + 1) * ch],
                    wtb[kt * 128 : (kt + 1) * 128, :],
                )
                nc.sync.dma_start(
                    xta_sb[:, kt * VS : (kt + 1) * VS],
                    xta[kt * 128 : (kt + 1) * 128, :],
                )
            nc.sync.dma_start(one_sb, xta[CK * 128 : CK * 128 + 1, :])

            for vm in range(KV):
                ps = psum.tile([128, ch], f32, tag="ps", name="ps_theta")
                for kt in range(CK):
                    nc.tensor.matmul(
                        ps,
                        lhsT=xta_sb[:, kt * VS + vm * 128 : kt * VS + (vm + 1) * 128],
                        rhs=wtb_sb[:, kt * ch : (kt + 1) * ch],
                        start=(kt == 0),
                        stop=False,
                    )
                nc.tensor.matmul(
                    ps,
                    lhsT=one_sb[:, vm * 128 : (vm + 1) * 128],
                    rhs=bias_sb,
                    start=False,
                    stop=True,
                )
                nc.vector.tensor_copy(xt_all[:, vm * ch : (vm + 1) * ch], ps)

        hs_pool = ctx.enter_context(tc.tile_pool(name="hs_pool", bufs=15))
        ht_pool = ctx.enter_context(tc.tile_pool(name="ht_pool", bufs=5))

        # ---- V2E partial GEMM + chunked AllReduce.
        # fp8e4 AllReduce payload: partial sums are |x| <~ 100 (well inside
        # the TRN e4m3 +-240 range); quantizing 8 partials at ~2.5% each adds
        # ~2.5%/sqrt(8) ~ 0.9% to the output, within the 2e-2 envelope. This
        # halves the collective's HBM/SDMA footprint, which otherwise starves
        # the concurrent GEMM's weight-panel streaming.
        arin = [
            dram.tile([chunks[c] * 128, ch], f8, name=f"arin{c}", tag=f"arin{c}")
            for c in range(nch)
        ]
        arout = [
            dram.tile([chunks[c] * 128, ch], f8, name=f"arout{c}", tag=f"arout{c}",
                      addr_space="Shared")
            for c in range(nch)
        ]
        xe_all = xe_pool.tile([128, EM * ch], f8)

        ht_pre = {}

        def load_ht(c, vm):
            t = ht_pool.tile([128, max(chunks) * 128], bf16, tag="ht", name="ht_sb")
            tv = t[:, 0 : chunks[c] * 128]
            nc.sync.dma_start(
                tv, htp[vm][:, cstart[c] * 128 : cstart[c + 1] * 128]
            )
            return tv

        for em in range(EM):
            hs_sb = hs_pool.tile([128, VS], bf16, tag="hs", name="hs_sb")
            nc.sync.dma_start(hs_sb, hsp[em])
            ps = psum.tile([128, ch], f32, tag="ps", name="ps_v2e")
            for kt in range(KV):
                nc.tensor.matmul(
                    ps,
                    lhsT=hs_sb[:, kt * 128 : (kt + 1) * 128],
                    rhs=xt_all[:, kt * ch : (kt + 1) * ch],
                    start=(kt == 0),
                    stop=(kt == KV - 1),
                )
            ar_sb = ar_pool.tile([128, ch], f8, tag="ar_sb", name="ar_sb")
            nc.vector.tensor_copy(ar_sb, ps)
            c = next(i for i in range(nch) if em < cstart[i + 1])
            j = em - cstart[c]
            nc.sync.dma_start(arin[c][j * 128 : (j + 1) * 128, :], ar_sb)
            if em == EM - 8:
                # Pre-issue the first two E2V weight-panel loads so they sit
                # in the SP FIFO just behind the remaining arin stores and
                # land before V2E drains.
                ht_pre[(0, 0)] = load_ht(0, 0)
                ht_pre[(0, 1)] = load_ht(0, 1)

            if j == chunks[c] - 1:
                nc.gpsimd.collective_compute(
                    "AllReduce",
                    mybir.AluOpType.add,
                    replica_groups=rg,
                    ins=[arin[c].opt()],
                    outs=[arout[c].opt()],
                )
                # ACT-ring DMAs: these wait on the collective, so they must
                # not sit in the SP-ring FIFO ahead of later arin/ht traffic.
                for jj in range(chunks[c]):
                    ke = cstart[c] + jj
                    nc.scalar.dma_start(
                        xe_all[:, ke * ch : (ke + 1) * ch],
                        arout[c][jj * 128 : (jj + 1) * 128, :],
                    )

        # ---- E2V GEMM (row-parallel), chunk-major over the AR chunks so
        # chunk c's compute overlaps AR chunk c+1. Per-vm fp32 accumulators
        # live in SBUF; the degree scalings are already folded into htp.
        # bf16 inter-chunk accumulator: rounds the chunk-0 partial once
        # (~0.3% of that partial), buying SBUF for the prefetch rings.
        acc = acc_pool.tile([128, KV * ch], bf16)
        for c in range(nch):
            last = c == nch - 1
            for vm in range(KV):
                ht_sb = ht_pre.pop((c, vm), None)
                if ht_sb is None:
                    ht_sb = load_ht(c, vm)
                ps = psum.tile([128, ch], f32, tag="ps2", name="ps_e2v")
                a = acc[:, vm * ch : (vm + 1) * ch]
                if c > 0:
                    # Seed the accumulation group with the prior chunks' sum
                    # (matmuls below run with start=False on top of it).
                    nc.vector.tensor_copy(ps, a)
                for kk in range(chunks[c]):
                    ke = cstart[c] + kk
                    nc.tensor.matmul(
                        ps,
                        lhsT=ht_sb[:, kk * 128 : (kk + 1) * 128],
                        rhs=xe_all[:, ke * ch : (ke + 1) * ch],
                        start=(kk == 0 and c == 0),
                        stop=(kk == chunks[c] - 1),
                        skip_group_check=(c > 0),
                    )
                if not last:
                    nc.vector.tensor_copy(a, ps)
                else:
                    o_sb = sb_out.tile([128, ch], f32, tag="o_sb", name="o_sb")
                    nc.scalar.activation(o_sb, ps, Relu)
                    # ACT ring: output stores must not block ht prefetch on SP.
                    nc.scalar.dma_start(out[vm * 128 : (vm + 1) * 128, :], o_sb)


def pack_inputs(X, H, W, b, cfg):
    """Host-side shard/cast/pack. Returns one input map per core."""
    from concurrent.futures import ThreadPoolExecutor

    n_v, n_e, ch, ncores = cfg["n_v"], cfg["n_e"], cfg["ch"], cfg["ncores"]
    VS = n_v // ncores
    KV = VS // 128
    EM = n_e // 128

    wtb = np.vstack(
        [
            np.ascontiguousarray(W.T).astype(np.float32),
            b[None, :].astype(np.float32),
        ]
    ).astype(BF16)

    # Degree scalings, folded into the E2V operand (matches the reference's
    # safe-reciprocal semantics: zero degree -> zero row/col scale).
    colsum = H.sum(axis=0, dtype=np.float32)
    rowsum = H.sum(axis=1, dtype=np.float32)
    de_inv = np.where(colsum == 0, 0.0, 1.0 / colsum).astype(np.float32)
    dv_inv = np.where(rowsum == 0, 0.0, 1.0 / rowsum).astype(np.float32)

    H_bf = H.astype(BF16)

    def pack_core(c):
        Hc = H_bf[c * VS : (c + 1) * VS]
        R = Hc.reshape(KV, 128, EM, 128)
        # hsp[em, p, kt*128+f] = Hc[kt*128+p, em*128+f]  (V2E lhsT panels)
        hsp = np.ascontiguousarray(R.transpose(2, 1, 0, 3)).reshape(EM, 128, VS)
        # htp[vm, p, ke*128+f] = Hs[vm*128+f, ke*128+p]  (E2V lhsT panels,
        # with both degree scalings pre-applied). The x4 compensates the
        # 1/4 pre-scale of xta that keeps the fp8 AllReduce in range.
        Hs = (
            (4.0 * dv_inv[c * VS : (c + 1) * VS, None])
            * H[c * VS : (c + 1) * VS]
            * de_inv[None, :]
        ).astype(BF16)
        R2 = Hs.reshape(KV, 128, EM, 128)
        htp = np.ascontiguousarray(R2.transpose(0, 3, 2, 1)).reshape(KV, 128, n_e)
        # 1/4 pre-scale (bias row included) so the fp8 AllReduce payload and
        # its intermediate ring sums stay inside TRN e4m3's +-240 range.
        Xc = X[c * VS : (c + 1) * VS]
        xta = np.vstack(
            [
                np.ascontiguousarray(Xc.T) * 0.25,
                np.full((1, VS), 0.25, np.float32),
            ]
        ).astype(BF16)
        return dict(hsp=hsp, htp=htp, xta=xta, wtb=wtb)

    with ThreadPoolExecutor(max_workers=ncores) as ex:
        return list(ex.map(pack_core, range(ncores)))


_cache = {}


def _build_compiled(cfg, reps=1):
    key = (tuple(sorted(cfg.items())), reps)
    if key in _cache:
        return _cache[key]
    from concourse import bacc, mybir, tile

    n_v, n_e, ch, ncores = cfg["n_v"], cfg["n_e"], cfg["ch"], cfg["ncores"]
    VS = n_v // ncores
    KV = VS // 128
    EM = n_e // 128

    nc = bacc.Bacc("TRN2", target_bir_lowering=False, debug=False,
                   num_devices=ncores)
    io = {
        "hsp": nc.dram_tensor("hsp", [EM, 128, VS], mybir.dt.bfloat16,
                              kind="ExternalInput").ap(),
        "htp": nc.dram_tensor("htp", [KV, 128, n_e], mybir.dt.bfloat16,
                              kind="ExternalInput").ap(),
        "xta": nc.dram_tensor("xta", [ch + 1, VS], mybir.dt.bfloat16,
                              kind="ExternalInput").ap(),
        "wtb": nc.dram_tensor("wtb", [ch + 1, ch], mybir.dt.bfloat16,
                              kind="ExternalInput").ap(),
        "out": nc.dram_tensor("out", [VS, ch], mybir.dt.float32,
                              kind="ExternalOutput").ap(),
    }
    with tile.TileContext(nc) as tc:
        for _ in range(reps):
            build_graph(tc, io, cfg)
    nc.compile()
    _cache[key] = nc
    return nc


def kernel(X, H, W, b, _trace=False, _cfg=None):
    from concourse.bass_utils import run_bass_kernel_spmd

    cfg = _cfg or _full_cfg()
    X = np.asarray(X, dtype=np.float32)
    H = np.asarray(H, dtype=np.float32)
    W = np.asarray(W, dtype=np.float32)
    b = np.asarray(b, dtype=np.float32)

    nc = _build_compiled(cfg)
    in_maps = pack_inputs(X, H, W, b, cfg)
    res = run_bass_kernel_spmd(
        nc, in_maps, core_ids=list(range(cfg["ncores"])), trace=_trace
    )
    kernel.last_result = res
    return np.concatenate([r["out"] for r in res.results], axis=0)


kernel.last_result = None


# revision 40
# speedup vs baseline: 1.0134x; 1.0134x over previous
"""Distributed HGNN+ convolution for 8 Trainium2 NeuronCores (Bass/Tile).

Math (dense hypergraph incidence H [N_V, N_E], features X [N_V, C]):
    Xt  = X @ W.T + b                    # theta
    Xe  = (H.T @ Xt) * 1/colsum(H)       # V2E mean aggregation
    Xv  = (H @ Xe)   * 1/rowsum(H)       # E2V mean aggregation
    out = relu(Xv)

Distribution: vertex rows are sharded across the 8 cores. Each core
computes theta on its vertex shard, a partial V2E GEMM, an AllReduce of
the partial edge features, then a fully row-parallel E2V GEMM over its
own vertex rows.

Both degree scalings are folded into the E2V operand on the host:
    htp2 = diag(4/rowsum) @ H @ diag(1/colsum)
so the device graph is three plain GEMMs + ReLU: every matmul streams a
full 512-wide PSUM bank and no on-device reciprocals/rescales exist.
(The 4x / 1/4 pair keeps the fp8 wire format in range, see below.)

The AllReduce runs in fp8e4 (half the SDMA/HBM footprint) in two
asymmetric chunks sized to hide it entirely: chunk 0 (40 edge tiles)
fires at ~62% of V2E and completes under the remaining V2E compute;
chunk 1 (24 tiles) fires at V2E end and completes under E2V's chunk-0
compute. E2V is chunk-major with a bf16 SBUF accumulator; the final
chunk seeds its PSUM accumulation group from the accumulator so the
epilogue is a single ReLU from PSUM.

During a concurrent collective, model-queue DMA throughput collapses
~10x, so both sides of the V2E pipeline carry deep SBUF rings (15 hs
input panels, 24 fp8 output staging tiles) to ride out the window.
GEMM compute is bf16 with fp32 PSUM accumulation; rel-err lands ~4.5e-3
against the fp32 reference, within the 2e-2 envelope.
"""

import contextlib

import numpy as np
import ml_dtypes

BF16 = ml_dtypes.bfloat16

# Problem shape (hardcoded per contract).
N_V, N_E, CH, NCORES = 16384, 8192, 512, 8


def _full_cfg():
    # Asymmetric AllReduce chunks (in 128-edge tiles): chunk 0 fires at
    # ~62% of V2E and hides under its tail; chunk 1 hides under E2V
    # chunk-0 compute (~170us vs ~55-115us collective).
    return dict(n_v=N_V, n_e=N_E, ch=CH, ncores=NCORES, chunks=(40, 24))


def build_graph(tc, io, cfg):
    """Emit the Tile IR. io: dict of DRAM APs: hsp, htp, xta, wtb, out."""
    from concourse import mybir

    nc = tc.nc
    f32 = mybir.dt.float32
    bf16 = mybir.dt.bfloat16
    f8 = mybir.dt.float8e4
    Relu = mybir.ActivationFunctionType.Relu

    n_v, n_e, ch, ncores = cfg["n_v"], cfg["n_e"], cfg["ch"], cfg["ncores"]
    chunks = list(cfg["chunks"])    # AR chunk sizes in 128-edge tiles
    nch = len(chunks)
    cstart = [sum(chunks[:i]) for i in range(nch + 1)]  # tile offsets
    VS = n_v // ncores      # vertices per core
    KV = VS // 128          # vertex 128-tiles per core
    EM = n_e // 128         # edge 128-tiles (global)
    CK = ch // 128          # theta contraction tiles over in-channels
    assert cstart[nch] == EM
    rg = [list(range(ncores))]

    hsp, htp, xta, wtb, out = io["hsp"], io["htp"], io["xta"], io["wtb"], io["out"]

    with contextlib.ExitStack() as ctx:
        xt_pool = ctx.enter_context(tc.tile_pool(name="xt_pool", bufs=1))
        xe_pool = ctx.enter_context(tc.tile_pool(name="xe_pool", bufs=1))
        acc_pool = ctx.enter_context(tc.tile_pool(name="acc_pool", bufs=1))
        sb_out = ctx.enter_context(tc.tile_pool(name="sb_out", bufs=2))
        # Deep fp8 staging ring: during a concurrent collective, model-queue
        # DMA throughput collapses ~10x, so V2E's arin stores crawl. The
        # ring lets ~24 edge tiles' outputs buffer in SBUF (cheap at 0.5KB/
        # partition) so the matmul stream never blocks on the store side.
        ar_pool = ctx.enter_context(tc.tile_pool(name="ar_pool", bufs=24))
        psum = ctx.enter_context(tc.tile_pool(name="psum", bufs=4, space="PSUM"))
        dram = ctx.enter_context(tc.tile_pool(name="dram", bufs=1, space="DRAM"))

        # Primer collective: a throwaway 2KB AllReduce fired at t~0 (input
        # is an unwritten DRAM tile; the result is discarded). It absorbs
        # the first-collective warm-up (~11us trigger delay + ~20us slower
        # execution) under the theta phase so the real chunk-0 AllReduce
        # behaves like a warm one.
        prime_in = dram.tile([128, 16], f8, name="prime_in", tag="prime_in")
        prime_out = dram.tile([128, 16], f8, name="prime_out", tag="prime_out",
                              addr_space="Shared")
        nc.gpsimd.collective_compute(
            "AllReduce",
            mybir.AluOpType.add,
            replica_groups=rg,
            ins=[prime_in.opt()],
            outs=[prime_out.opt()],
        )

        # ---- theta: Xt = X.T @ W.T + (0.25-ones) @ b, kept in SBUF as KV
        # tiles of [128 v, ch]. The bias is a rank-1 (K=1) matmul so no
        # zero-padded contraction tile is loaded. Per-k-tile DMAs so the
        # first theta matmul only waits for the first slices. The pool is
        # scoped to this phase so its SBUF is recycled into the hs ring.
        xt_all = xt_pool.tile([128, KV * ch], bf16)
        with tc.tile_pool(name="theta_in", bufs=1) as theta_in:
            xta_sb = theta_in.tile([128, CK * VS], bf16)
            wtb_sb = theta_in.tile([128, CK * ch], bf16)
            one_sb = theta_in.tile([1, VS], bf16)
            bias_sb = theta_in.tile([1, ch], bf16)
            nc.sync.dma_start(bias_sb, wtb[CK * 128 : CK * 128 + 1, :])
            nc.sync.dma_start(one_sb, xta[CK * 128 : CK * 128 + 1, :])
            for kt in range(CK):
                nc.sync.dma_start(
                    wtb_sb[:, kt * ch : (kt + 1) * ch],
                    wtb[kt * 128 : (kt + 1) * 128, :],
                )
                if kt == 0:
                    # Quarter-slices so the first matmul's dependency is
                    # ~128KB instead of the whole 512KB panel.
                    for q in range(4):
                        nc.sync.dma_start(
                            xta_sb[:, q * (VS // 4) : (q + 1) * (VS // 4)],
                            xta[0:128, q * (VS // 4) : (q + 1) * (VS // 4)],
                        )
                else:
                    nc.sync.dma_start(
                        xta_sb[:, kt * VS : (kt + 1) * VS],
                        xta[kt * 128 : (kt + 1) * 128, :],
                    )

            for vm in range(KV):
                ps = psum.tile([128, ch], f32, tag="ps", name="ps_theta")
                for kt in range(CK):
                    nc.tensor.matmul(
                        ps,
                        lhsT=xta_sb[:, kt * VS + vm * 128 : kt * VS + (vm + 1) * 128],
                        rhs=wtb_sb[:, kt * ch : (kt + 1) * ch],
                        start=(kt == 0),
                        stop=False,
                    )
                nc.tensor.matmul(
                    ps,
                    lhsT=one_sb[:, vm * 128 : (vm + 1) * 128],
                    rhs=bias_sb,
                    start=False,
                    stop=True,
                )
                nc.vector.tensor_copy(xt_all[:, vm * ch : (vm + 1) * ch], ps)

        hs_pool = ctx.enter_context(tc.tile_pool(name="hs_pool", bufs=15))
        ht_pool = ctx.enter_context(tc.tile_pool(name="ht_pool", bufs=5))

        # ---- V2E partial GEMM + chunked AllReduce.
        # fp8e4 AllReduce payload: partial sums are |x| <~ 100 (well inside
        # the TRN e4m3 +-240 range); quantizing 8 partials at ~2.5% each adds
        # ~2.5%/sqrt(8) ~ 0.9% to the output, within the 2e-2 envelope. This
        # halves the collective's HBM/SDMA footprint, which otherwise starves
        # the concurrent GEMM's weight-panel streaming.
        arin = [
            dram.tile([chunks[c] * 128, ch], f8, name=f"arin{c}", tag=f"arin{c}")
            for c in range(nch)
        ]
        arout = [
            dram.tile([chunks[c] * 128, ch], f8, name=f"arout{c}", tag=f"arout{c}",
                      addr_space="Shared")
            for c in range(nch)
        ]
        xe_all = xe_pool.tile([128, EM * ch], f8)

        ht_pre = {}

        def load_ht(c, vm):
            t = ht_pool.tile([128, max(chunks) * 128], bf16, tag="ht", name="ht_sb")
            tv = t[:, 0 : chunks[c] * 128]
            nc.sync.dma_start(
                tv, htp[vm][:, cstart[c] * 128 : cstart[c + 1] * 128]
            )
            return tv

        for em in range(EM):
            hs_sb = hs_pool.tile([128, VS], bf16, tag="hs", name="hs_sb")
            nc.sync.dma_start(hs_sb, hsp[em])
            ps = psum.tile([128, ch], f32, tag="ps", name="ps_v2e")
            for kt in range(KV):
                nc.tensor.matmul(
                    ps,
                    lhsT=hs_sb[:, kt * 128 : (kt + 1) * 128],
                    rhs=xt_all[:, kt * ch : (kt + 1) * ch],
                    start=(kt == 0),
                    stop=(kt == KV - 1),
                )
            ar_sb = ar_pool.tile([128, ch], f8, tag="ar_sb", name="ar_sb")
            nc.vector.tensor_copy(ar_sb, ps)
            c = next(i for i in range(nch) if em < cstart[i + 1])
            j = em - cstart[c]
            nc.sync.dma_start(arin[c][j * 128 : (j + 1) * 128, :], ar_sb)
            if em == EM - 8:
                # Pre-issue the first two E2V weight-panel loads so they sit
                # in the SP FIFO just behind the remaining arin stores and
                # land before V2E drains.
                ht_pre[(0, 0)] = load_ht(0, 0)
                ht_pre[(0, 1)] = load_ht(0, 1)

            if j == chunks[c] - 1:
                nc.gpsimd.collective_compute(
                    "AllReduce",
                    mybir.AluOpType.add,
                    replica_groups=rg,
                    ins=[arin[c].opt()],
                    outs=[arout[c].opt()],
                )
                # ACT-ring DMAs: these wait on the collective, so they must
                # not sit in the SP-ring FIFO ahead of later arin/ht traffic.
                for jj in range(chunks[c]):
                    ke = cstart[c] + jj
                    nc.scalar.dma_start(
                        xe_all[:, ke * ch : (ke + 1) * ch],
                        arout[c][jj * 128 : (jj + 1) * 128, :],
                    )

        # ---- E2V GEMM (row-parallel), chunk-major over the AR chunks so
        # chunk c's compute overlaps AR chunk c+1. Per-vm fp32 accumulators
        # live in SBUF; the degree scalings are already folded into htp.
        # bf16 inter-chunk accumulator: rounds the chunk-0 partial once
        # (~0.3% of that partial), buying SBUF for the prefetch rings.
        acc = acc_pool.tile([128, KV * ch], bf16)
        for c in range(nch):
            last = c == nch - 1
            for vm in range(KV):
                ht_sb = ht_pre.pop((c, vm), None)
                if ht_sb is None:
                    ht_sb = load_ht(c, vm)
                ps = psum.tile([128, ch], f32, tag="ps2", name="ps_e2v")
                a = acc[:, vm * ch : (vm + 1) * ch]
                if c > 0:
                    # Seed the accumulation group with the prior chunks' sum
                    # (matmuls below run with start=False on top of it).
                    nc.vector.tensor_copy(ps, a)
                for kk in range(chunks[c]):
                    ke = cstart[c] + kk
                    nc.tensor.matmul(
                        ps,
                        lhsT=ht_sb[:, kk * 128 : (kk + 1) * 128],
                        rhs=xe_all[:, ke * ch : (ke + 1) * ch],
                        start=(kk == 0 and c == 0),
                        stop=(kk == chunks[c] - 1),
                        skip_group_check=(c > 0),
                    )
                if not last:
                    nc.vector.tensor_copy(a, ps)
                else:
                    o_sb = sb_out.tile([128, ch], f32, tag="o_sb", name="o_sb")
                    nc.scalar.activation(o_sb, ps, Relu)
                    # ACT ring: output stores must not block ht prefetch on SP.
                    nc.scalar.dma_start(out[vm * 128 : (vm + 1) * 128, :], o_sb)


def pack_inputs(X, H, W, b, cfg):
    """Host-side shard/cast/pack. Returns one input map per core."""
    from concurrent.futures import ThreadPoolExecutor

    n_v, n_e, ch, ncores = cfg["n_v"], cfg["n_e"], cfg["ch"], cfg["ncores"]
    VS = n_v // ncores
    KV = VS // 128
    EM = n_e // 128

    wtb = np.vstack(
        [
            np.ascontiguousarray(W.T).astype(np.float32),
            b[None, :].astype(np.float32),
        ]
    ).astype(BF16)

    # Degree scalings, folded into the E2V operand (matches the reference's
    # safe-reciprocal semantics: zero degree -> zero row/col scale).
    colsum = H.sum(axis=0, dtype=np.float32)
    rowsum = H.sum(axis=1, dtype=np.float32)
    de_inv = np.where(colsum == 0, 0.0, 1.0 / colsum).astype(np.float32)
    dv_inv = np.where(rowsum == 0, 0.0, 1.0 / rowsum).astype(np.float32)

    H_bf = H.astype(BF16)

    def pack_core(c):
        Hc = H_bf[c * VS : (c + 1) * VS]
        R = Hc.reshape(KV, 128, EM, 128)
        # hsp[em, p, kt*128+f] = Hc[kt*128+p, em*128+f]  (V2E lhsT panels)
        hsp = np.ascontiguousarray(R.transpose(2, 1, 0, 3)).reshape(EM, 128, VS)
        # htp[vm, p, ke*128+f] = Hs[vm*128+f, ke*128+p]  (E2V lhsT panels,
        # with both degree scalings pre-applied). The x4 compensates the
        # 1/4 pre-scale of xta that keeps the fp8 AllReduce in range.
        Hs = (
            (4.0 * dv_inv[c * VS : (c + 1) * VS, None])
            * H[c * VS : (c + 1) * VS]
            * de_inv[None, :]
        ).astype(BF16)
        R2 = Hs.reshape(KV, 128, EM, 128)
        htp = np.ascontiguousarray(R2.transpose(0, 3, 2, 1)).reshape(KV, 128, n_e)
        # 1/4 pre-scale (bias row included) so the fp8 AllReduce payload and
        # its intermediate ring sums stay inside TRN e4m3's +-240 range.
        Xc = X[c * VS : (c + 1) * VS]
        xta = np.vstack(
            [
                np.ascontiguousarray(Xc.T) * 0.25,
                np.full((1, VS), 0.25, np.float32),
            ]
        ).astype(BF16)
        return dict(hsp=hsp, htp=htp, xta=xta, wtb=wtb)

    with ThreadPoolExecutor(max_workers=ncores) as ex:
        return list(ex.map(pack_core, range(ncores)))


_cache = {}


def _build_compiled(cfg, reps=1):
    key = (tuple(sorted(cfg.items())), reps)
    if key in _cache:
        return _cache[key]
    from concourse import bacc, mybir, tile

    n_v, n_e, ch, ncores = cfg["n_v"], cfg["n_e"], cfg["ch"], cfg["ncores"]
    VS = n_v // ncores
    KV = VS // 128
    EM = n_e // 128

    nc = bacc.Bacc("TRN2", target_bir_lowering=False, debug=False,
                   num_devices=ncores)
    io = {
        "hsp": nc.dram_tensor("hsp", [EM, 128, VS], mybir.dt.bfloat16,
                              kind="ExternalInput").ap(),
        "htp": nc.dram_tensor("htp", [KV, 128, n_e], mybir.dt.bfloat16,
                              kind="ExternalInput").ap(),
        "xta": nc.dram_tensor("xta", [ch + 1, VS], mybir.dt.bfloat16,
                              kind="ExternalInput").ap(),
        "wtb": nc.dram_tensor("wtb", [ch + 1, ch], mybir.dt.bfloat16,
                              kind="ExternalInput").ap(),
        "out": nc.dram_tensor("out", [VS, ch], mybir.dt.float32,
                              kind="ExternalOutput").ap(),
    }
    with tile.TileContext(nc) as tc:
        for _ in range(reps):
            build_graph(tc, io, cfg)
    nc.compile()
    _cache[key] = nc
    return nc


def kernel(X, H, W, b, _trace=False, _cfg=None):
    from concourse.bass_utils import run_bass_kernel_spmd

    cfg = _cfg or _full_cfg()
    X = np.asarray(X, dtype=np.float32)
    H = np.asarray(H, dtype=np.float32)
    W = np.asarray(W, dtype=np.float32)
    b = np.asarray(b, dtype=np.float32)

    nc = _build_compiled(cfg)
    in_maps = pack_inputs(X, H, W, b, cfg)
    res = run_bass_kernel_spmd(
        nc, in_maps, core_ids=list(range(cfg["ncores"])), trace=_trace
    )
    kernel.last_result = res
    return np.concatenate([r["out"] for r in res.results], axis=0)


kernel.last_result = None


# revision 44
# speedup vs baseline: 1.0297x; 1.0161x over previous
"""Distributed HGNN+ convolution for 8 Trainium2 NeuronCores (Bass/Tile).

Math (dense hypergraph incidence H [N_V, N_E], features X [N_V, C]):
    Xt  = X @ W.T + b                    # theta
    Xe  = (H.T @ Xt) * 1/colsum(H)       # V2E mean aggregation
    Xv  = (H @ Xe)   * 1/rowsum(H)       # E2V mean aggregation
    out = relu(Xv)

Distribution: vertex rows are sharded across the 8 cores. Each core
computes theta on its vertex shard, a partial V2E GEMM, an AllReduce of
the partial edge features, then a fully row-parallel E2V GEMM over its
own vertex rows.

Both degree scalings are folded into the E2V operand on the host:
    htp2 = diag(4/rowsum) @ H @ diag(1/colsum)
so the device graph is three plain GEMMs + ReLU: every matmul streams a
full 512-wide PSUM bank and no on-device reciprocals/rescales exist.
(The 4x / 1/4 pair keeps the fp8 wire format in range, see below.)

The AllReduce runs in fp8e4 (half the SDMA/HBM footprint) in two
asymmetric chunks sized to hide it entirely: chunk 0 (40 edge tiles)
fires at ~62% of V2E and completes under the remaining V2E compute;
chunk 1 (24 tiles) fires at V2E end and completes under E2V's chunk-0
compute. E2V is chunk-major with a bf16 SBUF accumulator; the final
chunk seeds its PSUM accumulation group from the accumulator so the
epilogue is a single ReLU from PSUM.

During a concurrent collective, model-queue DMA throughput collapses
~10x, so both sides of the V2E pipeline carry deep SBUF rings (15 hs
input panels, 24 fp8 output staging tiles) to ride out the window.
GEMM compute is bf16 with fp32 PSUM accumulation; rel-err lands ~4.5e-3
against the fp32 reference, within the 2e-2 envelope.
"""

import contextlib

import numpy as np
import ml_dtypes

BF16 = ml_dtypes.bfloat16

# Problem shape (hardcoded per contract).
N_V, N_E, CH, NCORES = 16384, 8192, 512, 8


def _full_cfg():
    # Asymmetric AllReduce chunks (in 128-edge tiles): chunk 0 fires at
    # ~62% of V2E and hides under its tail; chunk 1 hides under E2V
    # chunk-0 compute (~170us vs ~55-115us collective).
    return dict(n_v=N_V, n_e=N_E, ch=CH, ncores=NCORES, chunks=(40, 24))


def build_graph(tc, io, cfg):
    """Emit the Tile IR. io: dict of DRAM APs: hsp, htp, xta, wtb, out."""
    from concourse import mybir

    nc = tc.nc
    f32 = mybir.dt.float32
    bf16 = mybir.dt.bfloat16
    f8 = mybir.dt.float8e4
    Relu = mybir.ActivationFunctionType.Relu
    Copy = mybir.ActivationFunctionType.Copy

    n_v, n_e, ch, ncores = cfg["n_v"], cfg["n_e"], cfg["ch"], cfg["ncores"]
    chunks = list(cfg["chunks"])    # AR chunk sizes in 128-edge tiles
    nch = len(chunks)
    cstart = [sum(chunks[:i]) for i in range(nch + 1)]  # tile offsets
    VS = n_v // ncores      # vertices per core
    KV = VS // 128          # vertex 128-tiles per core
    EM = n_e // 128         # edge 128-tiles (global)
    CK = ch // 128          # theta contraction tiles over in-channels
    assert cstart[nch] == EM
    rg = [list(range(ncores))]

    hsp, htp, xta, wtb, out = io["hsp"], io["htp"], io["xta"], io["wtb"], io["out"]

    with contextlib.ExitStack() as ctx:
        xt_pool = ctx.enter_context(tc.tile_pool(name="xt_pool", bufs=1))
        xe_pool = ctx.enter_context(tc.tile_pool(name="xe_pool", bufs=1))
        acc_pool = ctx.enter_context(tc.tile_pool(name="acc_pool", bufs=1))
        sb_out = ctx.enter_context(tc.tile_pool(name="sb_out", bufs=2))
        # Deep fp8 staging ring: during a concurrent collective, model-queue
        # DMA throughput collapses ~10x, so V2E's arin stores crawl. The
        # ring lets ~24 edge tiles' outputs buffer in SBUF (cheap at 0.5KB/
        # partition) so the matmul stream never blocks on the store side.
        ar_pool = ctx.enter_context(tc.tile_pool(name="ar_pool", bufs=24))
        # Separate PSUM pools: 5 rotation banks for the theta/V2E stream
        # (the deeper rotation decouples the matmul stream from the
        # psum-evacuation latency), 3 for E2V.
        psum = ctx.enter_context(tc.tile_pool(name="psum", bufs=5, space="PSUM"))
        psum_e = ctx.enter_context(tc.tile_pool(name="psum_e", bufs=3, space="PSUM"))
        dram = ctx.enter_context(tc.tile_pool(name="dram", bufs=1, space="DRAM"))

        # Primer collective: a throwaway 2KB AllReduce fired at t~0 (input
        # is an unwritten DRAM tile; the result is discarded). It absorbs
        # the first-collective warm-up (~11us trigger delay + ~20us slower
        # execution) under the theta phase so the real chunk-0 AllReduce
        # behaves like a warm one.
        prime_in = dram.tile([128, 16], f8, name="prime_in", tag="prime_in")
        prime_out = dram.tile([128, 16], f8, name="prime_out", tag="prime_out",
                              addr_space="Shared")
        nc.gpsimd.collective_compute(
            "AllReduce",
            mybir.AluOpType.add,
            replica_groups=rg,
            ins=[prime_in.opt()],
            outs=[prime_out.opt()],
        )

        # ---- theta: Xt = X.T @ W.T + (0.25-ones) @ b, kept in SBUF as KV
        # tiles of [128 v, ch]. The bias is a rank-1 (K=1) matmul so no
        # zero-padded contraction tile is loaded. Per-k-tile DMAs so the
        # first theta matmul only waits for the first slices. The pool is
        # scoped to this phase so its SBUF is recycled into the hs ring.
        xt_all = xt_pool.tile([128, KV * ch], bf16)
        with tc.tile_pool(name="theta_in", bufs=1) as theta_in:
            xta_sb = theta_in.tile([128, CK * VS], bf16)
            wtb_sb = theta_in.tile([128, CK * ch], bf16)
            one_sb = theta_in.tile([1, VS], bf16)
            bias_sb = theta_in.tile([1, ch], bf16)
            nc.sync.dma_start(bias_sb, wtb[CK * 128 : CK * 128 + 1, :])
            nc.sync.dma_start(one_sb, xta[CK * 128 : CK * 128 + 1, :])
            for kt in range(CK):
                nc.sync.dma_start(
                    wtb_sb[:, kt * ch : (kt + 1) * ch],
                    wtb[kt * 128 : (kt + 1) * 128, :],
                )
                if kt == 0:
                    # Quarter-slices so the first matmul's dependency is
                    # ~128KB instead of the whole 512KB panel.
                    for q in range(4):
                        nc.sync.dma_start(
                            xta_sb[:, q * (VS // 4) : (q + 1) * (VS // 4)],
                            xta[0:128, q * (VS // 4) : (q + 1) * (VS // 4)],
                        )
                else:
                    nc.sync.dma_start(
                        xta_sb[:, kt * VS : (kt + 1) * VS],
                        xta[kt * 128 : (kt + 1) * 128, :],
                    )

            for vm in range(KV):
                ps = psum.tile([128, ch], f32, tag="ps", name="ps_theta")
                for kt in range(CK):
                    nc.tensor.matmul(
                        ps,
                        lhsT=xta_sb[:, kt * VS + vm * 128 : kt * VS + (vm + 1) * 128],
                        rhs=wtb_sb[:, kt * ch : (kt + 1) * ch],
                        start=(kt == 0),
                        stop=False,
                    )
                nc.tensor.matmul(
                    ps,
                    lhsT=one_sb[:, vm * 128 : (vm + 1) * 128],
                    rhs=bias_sb,
                    start=False,
                    stop=True,
                )
                # Evacuate on the (idle) scalar engine: DVE casts were the
                # theta pipeline's rate limiter.
                nc.scalar.activation(
                    xt_all[:, vm * ch : (vm + 1) * ch], ps, Copy
                )

        hs_pool = ctx.enter_context(tc.tile_pool(name="hs_pool", bufs=15))
        ht_pool = ctx.enter_context(tc.tile_pool(name="ht_pool", bufs=5))

        # ---- V2E partial GEMM + chunked AllReduce.
        # fp8e4 AllReduce payload: partial sums are |x| <~ 100 (well inside
        # the TRN e4m3 +-240 range); quantizing 8 partials at ~2.5% each adds
        # ~2.5%/sqrt(8) ~ 0.9% to the output, within the 2e-2 envelope. This
        # halves the collective's HBM/SDMA footprint, which otherwise starves
        # the concurrent GEMM's weight-panel streaming.
        arin = [
            dram.tile([chunks[c] * 128, ch], f8, name=f"arin{c}", tag=f"arin{c}")
            for c in range(nch)
        ]
        arout = [
            dram.tile([chunks[c] * 128, ch], f8, name=f"arout{c}", tag=f"arout{c}",
                      addr_space="Shared")
            for c in range(nch)
        ]
        xe_all = xe_pool.tile([128, EM * ch], f8)

        ht_pre = {}

        def load_ht(c, vm):
            t = ht_pool.tile([128, max(chunks) * 128], bf16, tag="ht", name="ht_sb")
            tv = t[:, 0 : chunks[c] * 128]
            nc.sync.dma_start(
                tv, htp[vm][:, cstart[c] * 128 : cstart[c + 1] * 128]
            )
            return tv

        for em in range(EM):
            hs_sb = hs_pool.tile([128, VS], bf16, tag="hs", name="hs_sb")
            nc.sync.dma_start(hs_sb, hsp[em])
            ps = psum.tile([128, ch], f32, tag="ps", name="ps_v2e")
            for kt in range(KV):
                nc.tensor.matmul(
                    ps,
                    lhsT=hs_sb[:, kt * 128 : (kt + 1) * 128],
                    rhs=xt_all[:, kt * ch : (kt + 1) * ch],
                    start=(kt == 0),
                    stop=(kt == KV - 1),
                )
            ar_sb = ar_pool.tile([128, ch], f8, tag="ar_sb", name="ar_sb")
            nc.vector.tensor_copy(ar_sb, ps)
            c = next(i for i in range(nch) if em < cstart[i + 1])
            j = em - cstart[c]
            nc.sync.dma_start(arin[c][j * 128 : (j + 1) * 128, :], ar_sb)
            if em == EM - 8:
                # Pre-issue the first two E2V weight-panel loads so they sit
                # in the SP FIFO just behind the remaining arin stores and
                # land before V2E drains.
                ht_pre[(0, 0)] = load_ht(0, 0)
                ht_pre[(0, 1)] = load_ht(0, 1)

            if j == chunks[c] - 1:
                nc.gpsimd.collective_compute(
                    "AllReduce",
                    mybir.AluOpType.add,
                    replica_groups=rg,
                    ins=[arin[c].opt()],
                    outs=[arout[c].opt()],
                )
                # ACT-ring DMAs: these wait on the collective, so they must
                # not sit in the SP-ring FIFO ahead of later arin/ht traffic.
                for jj in range(chunks[c]):
                    ke = cstart[c] + jj
                    nc.scalar.dma_start(
                        xe_all[:, ke * ch : (ke + 1) * ch],
                        arout[c][jj * 128 : (jj + 1) * 128, :],
                    )

        # ---- E2V GEMM (row-parallel), chunk-major over the AR chunks so
        # chunk c's compute overlaps AR chunk c+1. Per-vm fp32 accumulators
        # live in SBUF; the degree scalings are already folded into htp.
        # bf16 inter-chunk accumulator: rounds the chunk-0 partial once
        # (~0.3% of that partial), buying SBUF for the prefetch rings.
        acc = acc_pool.tile([128, KV * ch], bf16)
        for c in range(nch):
            last = c == nch - 1
            for vm in range(KV):
                ht_sb = ht_pre.pop((c, vm), None)
                if ht_sb is None:
                    ht_sb = load_ht(c, vm)
                ps = psum_e.tile([128, ch], f32, tag="ps2", name="ps_e2v")
                a = acc[:, vm * ch : (vm + 1) * ch]
                if c > 0:
                    # Seed the accumulation group with the prior chunks' sum
                    # (matmuls below run with start=False on top of it).
                    nc.vector.tensor_copy(ps, a)
                for kk in range(chunks[c]):
                    ke = cstart[c] + kk
                    nc.tensor.matmul(
                        ps,
                        lhsT=ht_sb[:, kk * 128 : (kk + 1) * 128],
                        rhs=xe_all[:, ke * ch : (ke + 1) * ch],
                        start=(kk == 0 and c == 0),
                        stop=(kk == chunks[c] - 1),
                        skip_group_check=(c > 0),
                    )
                if not last:
                    nc.vector.tensor_copy(a, ps)
                else:
                    o_sb = sb_out.tile([128, ch], f32, tag="o_sb", name="o_sb")
                    nc.scalar.activation(o_sb, ps, Relu)
                    # ACT ring: output stores must not block ht prefetch on SP.
                    nc.scalar.dma_start(out[vm * 128 : (vm + 1) * 128, :], o_sb)


def pack_inputs(X, H, W, b, cfg):
    """Host-side shard/cast/pack. Returns one input map per core."""
    from concurrent.futures import ThreadPoolExecutor

    n_v, n_e, ch, ncores = cfg["n_v"], cfg["n_e"], cfg["ch"], cfg["ncores"]
    VS = n_v // ncores
    KV = VS // 128
    EM = n_e // 128

    wtb = np.vstack(
        [
            np.ascontiguousarray(W.T).astype(np.float32),
            b[None, :].astype(np.float32),
        ]
    ).astype(BF16)

    # Degree scalings, folded into the E2V operand (matches the reference's
    # safe-reciprocal semantics: zero degree -> zero row/col scale).
    colsum = H.sum(axis=0, dtype=np.float32)
    rowsum = H.sum(axis=1, dtype=np.float32)
    de_inv = np.where(colsum == 0, 0.0, 1.0 / colsum).astype(np.float32)
    dv_inv = np.where(rowsum == 0, 0.0, 1.0 / rowsum).astype(np.float32)

    H_bf = H.astype(BF16)

    def pack_core(c):
        Hc = H_bf[c * VS : (c + 1) * VS]
        R = Hc.reshape(KV, 128, EM, 128)
        # hsp[em, p, kt*128+f] = Hc[kt*128+p, em*128+f]  (V2E lhsT panels)
        hsp = np.ascontiguousarray(R.transpose(2, 1, 0, 3)).reshape(EM, 128, VS)
        # htp[vm, p, ke*128+f] = Hs[vm*128+f, ke*128+p]  (E2V lhsT panels,
        # with both degree scalings pre-applied). The x4 compensates the
        # 1/4 pre-scale of xta that keeps the fp8 AllReduce in range.
        Hs = (
            (4.0 * dv_inv[c * VS : (c + 1) * VS, None])
            * H[c * VS : (c + 1) * VS]
            * de_inv[None, :]
        ).astype(BF16)
        R2 = Hs.reshape(KV, 128, EM, 128)
        htp = np.ascontiguousarray(R2.transpose(0, 3, 2, 1)).reshape(KV, 128, n_e)
        # 1/4 pre-scale (bias row included) so the fp8 AllReduce payload and
        # its intermediate ring sums stay inside TRN e4m3's +-240 range.
        Xc = X[c * VS : (c + 1) * VS]
        xta = np.vstack(
            [
                np.ascontiguousarray(Xc.T) * 0.25,
                np.full((1, VS), 0.25, np.float32),
            ]
        ).astype(BF16)
        return dict(hsp=hsp, htp=htp, xta=xta, wtb=wtb)

    with ThreadPoolExecutor(max_workers=ncores) as ex:
        return list(ex.map(pack_core, range(ncores)))


_cache = {}


def _build_compiled(cfg, reps=1):
    key = (tuple(sorted(cfg.items())), reps)
    if key in _cache:
        return _cache[key]
    from concourse import bacc, mybir, tile

    n_v, n_e, ch, ncores = cfg["n_v"], cfg["n_e"], cfg["ch"], cfg["ncores"]
    VS = n_v // ncores
    KV = VS // 128
    EM = n_e // 128

    nc = bacc.Bacc("TRN2", target_bir_lowering=False, debug=False,
                   num_devices=ncores)
    io = {
        "hsp": nc.dram_tensor("hsp", [EM, 128, VS], mybir.dt.bfloat16,
                              kind="ExternalInput").ap(),
        "htp": nc.dram_tensor("htp", [KV, 128, n_e], mybir.dt.bfloat16,
                              kind="ExternalInput").ap(),
        "xta": nc.dram_tensor("xta", [ch + 1, VS], mybir.dt.bfloat16,
                              kind="ExternalInput").ap(),
        "wtb": nc.dram_tensor("wtb", [ch + 1, ch], mybir.dt.bfloat16,
                              kind="ExternalInput").ap(),
        "out": nc.dram_tensor("out", [VS, ch], mybir.dt.float32,
                              kind="ExternalOutput").ap(),
    }
    with tile.TileContext(nc) as tc:
        for _ in range(reps):
            build_graph(tc, io, cfg)
    nc.compile()
    _cache[key] = nc
    return nc


def kernel(X, H, W, b, _trace=False, _cfg=None):
    from concourse.bass_utils import run_bass_kernel_spmd

    cfg = _cfg or _full_cfg()
    X = np.asarray(X, dtype=np.float32)
    H = np.asarray(H, dtype=np.float32)
    W = np.asarray(W, dtype=np.float32)
    b = np.asarray(b, dtype=np.float32)

    nc = _build_compiled(cfg)
    in_maps = pack_inputs(X, H, W, b, cfg)
    res = run_bass_kernel_spmd(
        nc, in_maps, core_ids=list(range(cfg["ncores"])), trace=_trace
    )
    kernel.last_result = res
    return np.concatenate([r["out"] for r in res.results], axis=0)


kernel.last_result = None
